# revision 9
# baseline (speedup 1.0000x reference)
"""Trainium2 Bass kernel for a 5-member ensemble dynamics MLP.

Model: per ensemble e, x[e] @ w0[e]+b0 -> silu -> (200x200 silu) x3 ->
w4[e]+b4 -> split (mean, logvar) -> double softplus clamp of logvar.

Sharding: pure data parallel over the batch dim (65536 -> 8 x 8192);
the ~1.4 MB of ensemble weights is replicated to every core.

v4 layout notes:
- All matmul operands are bfloat16; PSUM accumulates fp32. bf16 streams
  the PE at 1 cycle/row (fp32r is ~2x slower on HW).
- NT=512 with ONE merged [128, 1024] PSUM tile per layer: columns 0:512
  hold the M-block 0:128, columns 512:1024 hold features 128:200 (the
  stationary is zero-padded to M=128 so every PSUM row is written).
  One layer therefore costs one bias-free Silu over [128, 1024] on the
  scalar engine, and the 2-bank tiles give the psum pool 4 generations
  -> the PE runs ~2 layers ahead of the scalar engine and its p-state
  (1.2 -> 2.4 GHz after ~3us continuous) survives tile boundaries.
- All biases ride inside the matmuls: x is packed with a ones row, and
  each layer's K-block-b stationary carries [weights; bias row]. The
  ones lane regenerates itself through Silu via a weight v* with
  silu(v*) = 1 placed on the (ones-in -> ones-out) diagonal element.
- Output layer: W4' = [mean(31) | pad | logvar(31)] (single M=63 block,
  bias row included), so mean is a plain copy of PSUM 0:31 and raw
  logvar sits at 32:63 (both 32-aligned).
- DMA discipline: descriptor generation occupies the issuing queue, so
  inputs ride the sync queue (w0 -> x -> rest, per ensemble, so tile 0
  starts within ~5us) and outputs ride the vector queue (ordered after
  the DVE ops that produce them; no head-of-line blocking of the next
  ensemble's loads).
- logvar clamp (phase 2) uses the exact identity
    out = min + ln(C2 + t) - ln(1 + t),  t = e^{max - lv},
    C2 = 1 + e^{max - min}
  (one Exp + two Ln, one table set, + one DVE op), emitted inline per
  ensemble so the program-order scheduler keeps each run contiguous
  (~3 act-table switches per ensemble boundary).
- Raw logvar rows are staged packed 4-tiles-up (31 rows per 32-stride
  group) so phase-2 activations run ~124/128 full partitions.
"""

import sys

if "/opt/trn_rl_repo" not in sys.path:
    sys.path.insert(0, "/opt/trn_rl_repo")

import numpy as np

E = 5
B = 65536
IN_DIM = 38
INP = IN_DIM + 1  # +1 ones row for bias
H = 200
OUT = 31  # mean / logvar feature count
NCORES = 8
BS = B // NCORES  # samples per core
NT = 512  # batch-tile columns
NTILES = BS // NT
K0 = 128
K1 = H - K0 + 1  # 73: features 128:200 + ones/bias row
M4 = 2 * OUT + 1  # packed L4 output block: mean | pad | logvar
PACK = 4  # logvar tiles packed per partition group in phase 2
RSTRIDE = 32  # partition stride per packed tile
P2P = PACK * RSTRIDE  # 128 partitions, top row of each 32-group unused
P2N = 1024  # phase-2 Ln/DVE free-dim chunk
# silu(VSTAR) == 1.0: the ones lane regenerates itself through each layer
VSTAR = 1.2784645

_CACHE = {}


def _build():
    import concourse.bass as bass  # noqa: F401
    import concourse.tile as tile
    from concourse import bacc, mybir
    from contextlib import ExitStack

    fp32 = mybir.dt.float32
    bf16 = mybir.dt.bfloat16
    AF = mybir.ActivationFunctionType
    ALU = mybir.AluOpType

    nc = bacc.Bacc("TRN2", target_bir_lowering=False, debug=False)

    xT = nc.dram_tensor("xT", [E, INP, BS], bf16, kind="ExternalInput").ap()
    # stationary blocks, host-packed (bias rows + ones-regen included):
    #   wa[l]: [E, 128, 256] = K-block 0:128   -> [Ma(128) | Mb(128, padded)]
    #   wb[l]: [E, 73, 256]  = K-block 128:201 -> [Ma(128) | Mb(128, padded)]
    w0_d = nc.dram_tensor("w0p", [E, INP, 256], bf16, kind="ExternalInput").ap()
    wa_d = [
        nc.dram_tensor(f"w{l}a", [E, K0, 256], bf16, kind="ExternalInput").ap()
        for l in (1, 2, 3)
    ]
    wb_d = [
        nc.dram_tensor(f"w{l}b", [E, K1, 256], bf16, kind="ExternalInput").ap()
        for l in (1, 2, 3)
    ]
    w4a_d = nc.dram_tensor("w4a", [E, K0, M4], bf16, kind="ExternalInput").ap()
    w4b_d = nc.dram_tensor("w4b", [E, K1, M4], bf16, kind="ExternalInput").ap()
    # phase-2 per-partition constants, pre-tiled to the packed 128 rows
    c1_d = nc.dram_tensor("c1", [P2P, 1], fp32, kind="ExternalInput").ap()
    c2_d = nc.dram_tensor("c2", [P2P, 1], fp32, kind="ExternalInput").ap()
    minlv_d = nc.dram_tensor("minlv", [P2P, 1], fp32, kind="ExternalInput").ap()
    om_d = nc.dram_tensor("out_mean", [E, OUT, BS], fp32, kind="ExternalOutput").ap()
    ol_d = nc.dram_tensor("out_logvar", [E, OUT, BS], fp32, kind="ExternalOutput").ap()

    with tile.TileContext(nc) as tc, ExitStack() as ctx:
        wpool = ctx.enter_context(tc.tile_pool(name="wts", bufs=1))
        stpool = ctx.enter_context(tc.tile_pool(name="stage", bufs=1))
        xpool = ctx.enter_context(tc.tile_pool(name="x", bufs=2))
        hpool = ctx.enter_context(tc.tile_pool(name="h", bufs=4))
        pspool = ctx.enter_context(tc.tile_pool(name="ps", bufs=4, space="PSUM"))
        mpool = ctx.enter_context(tc.tile_pool(name="mean", bufs=1))
        tpool = ctx.enter_context(tc.tile_pool(name="p2t", bufs=2))
        p2pool = ctx.enter_context(tc.tile_pool(name="p2", bufs=2))

        W = {}

        def _const(tag, shape, src, dt=fp32):
            t = wpool.tile(shape, dt, tag=tag)
            nc.sync.dma_start(t[:], src)
            W[tag] = t
            return t

        # global phase-2 constants
        c1 = _const("c1", [P2P, 1], c1_d[:])
        c2 = _const("c2", [P2P, 1], c2_d[:])
        minlv = _const("minlv", [P2P, 1], minlv_d[:])

        # raw-logvar staging buffers, one per ensemble, packed 4-tiles-up
        stage = []
        for e in range(E):
            st = stpool.tile(
                [P2P, NTILES // PACK * NT], fp32, tag=f"stage_{e}", name=f"stage_{e}"
            )
            nc.vector.memset(st[:], 0.0)
            stage.append(st)

        ncol = NTILES // PACK * NT  # staged cols per ensemble

        def load_ensemble(e):
            """Queue ensemble e's input DMAs (sync ring), w0+x first."""
            _const(f"w0_{e}", [INP, 256], w0_d[e], bf16)
            xe = xpool.tile([INP, BS], bf16, tag="x")
            nc.sync.dma_start(xe[:], xT[e])
            for l in (1, 2, 3):
                _const(f"w{l}a_{e}", [K0, 256], wa_d[l - 1][e], bf16)
                _const(f"w{l}b_{e}", [K1, 256], wb_d[l - 1][e], bf16)
            _const(f"w4a_{e}", [K0, M4], w4a_d[e], bf16)
            _const(f"w4b_{e}", [K1, M4], w4b_d[e], bf16)
            return xe

        xe_next = load_ensemble(0)
        for e in range(E):
            xe = xe_next
            meanbuf = mpool.tile([OUT, BS], fp32, tag="meanbuf")

            # ---- MLP tiles (Silu table) ----
            for t in range(NTILES):
                cs = slice(t * NT, (t + 1) * NT)

                # layer 0: one merged [128, 2*NT] psum tile (Ma | Mb)
                ps = pspool.tile([K0, 2 * NT], fp32, tag="ps")
                w0e = W[f"w0_{e}"]
                nc.tensor.matmul(
                    ps[:, 0:NT], w0e[:, 0:128], xe[:, cs], start=True, stop=True
                )
                nc.tensor.matmul(
                    ps[:, NT : 2 * NT], w0e[:, 128:256], xe[:, cs], start=True, stop=True
                )
                h = hpool.tile([K0, 2 * NT], bf16, tag="h")
                nc.scalar.activation(h[:], ps[:], AF.Silu)

                # layers 1..3
                for l in (1, 2, 3):
                    wa, wb = W[f"w{l}a_{e}"], W[f"w{l}b_{e}"]
                    ha, hb = h[0:K0, 0:NT], h[0:K1, NT : 2 * NT]
                    ps = pspool.tile([K0, 2 * NT], fp32, tag="ps")
                    nc.tensor.matmul(
                        ps[:, 0:NT], wa[:, 0:128], ha, start=True, stop=False
                    )
                    nc.tensor.matmul(
                        ps[:, 0:NT], wb[:, 0:128], hb, start=False, stop=True
                    )
                    nc.tensor.matmul(
                        ps[:, NT : 2 * NT], wa[:, 128:256], ha, start=True, stop=False
                    )
                    nc.tensor.matmul(
                        ps[:, NT : 2 * NT], wb[:, 128:256], hb, start=False, stop=True
                    )
                    h = hpool.tile([K0, 2 * NT], bf16, tag="h")
                    nc.scalar.activation(h[:], ps[:], AF.Silu)

                # layer 4: single packed M=63 block (mean | pad | logvar)
                ha, hb = h[0:K0, 0:NT], h[0:K1, NT : 2 * NT]
                pm = pspool.tile([M4, NT], fp32, tag="ps")
                nc.tensor.matmul(pm[:], W[f"w4a_{e}"][:], ha, start=True, stop=False)
                nc.tensor.matmul(pm[:], W[f"w4b_{e}"][:], hb, start=False, stop=True)

                nc.vector.tensor_copy(meanbuf[:, cs], pm[0:OUT, :])

                # stash raw logvar rows: tile t -> rows 32*(t%4), cols NT*(t//4)
                r = (t % PACK) * RSTRIDE
                c = (t // PACK) * NT
                nc.vector.tensor_copy(
                    stage[e][r : r + OUT, c : c + NT], pm[OUT + 1 : M4, :]
                )

            # prefetch next ensemble's inputs into the sync ring BEFORE this
            # ensemble's outputs, so the late-ready output DMAs can't
            # head-of-line-block the loads
            if e + 1 < E:
                xe_next = load_ensemble(e + 1)
            nc.sync.dma_start(om_d[e], meanbuf[:])

            # ---- logvar clamp (Exp/Ln table), inline per ensemble ----
            #   t   = Exp(-z + c1) = e^{max - lv}   (c1 = max; b4lv in matmul)
            #   out = min + Ln(t + C2) - Ln(t + 1),  C2 = 1 + e^{max - min}
            te = tpool.tile([P2P, ncol], fp32, tag="p2t")
            nc.scalar.activation(te[:], stage[e][:], AF.Exp, bias=c1[:], scale=-1.0)
            for g in range(ncol // P2N):
                gs = slice(g * P2N, (g + 1) * P2N)
                a = p2pool.tile([P2P, P2N], fp32, tag="p2a")
                nc.scalar.activation(a[:], te[:, gs], AF.Ln, bias=c2[:])
                b = p2pool.tile([P2P, P2N], fp32, tag="p2b")
                nc.scalar.activation(b[:], te[:, gs], AF.Ln, bias=1.0)
                lvo = p2pool.tile([P2P, P2N], fp32, tag="p2c")
                # (a + min) - b
                nc.vector.scalar_tensor_tensor(
                    lvo[:], a[:], minlv[:], b[:], ALU.add, ALU.subtract
                )
                # unpack to [E, 31, BS] via the vector DMA queue
                for j in range(P2N // NT):
                    tcol = g * (P2N // NT) + j  # global col-block = t // PACK
                    for r in range(PACK):
                        t = tcol * PACK + r
                        nc.sync.dma_start(
                            ol_d[e, :, t * NT : (t + 1) * NT],
                            lvo[r * RSTRIDE : r * RSTRIDE + OUT, j * NT : (j + 1) * NT],
                        )

    nc.compile()
    return nc


def _prep_host(x, w0, b0, w1, b1, w2, b2, w3, b3, w4, b4, max_logvar, min_logvar):
    import ml_dtypes

    f = np.float32
    bf = ml_dtypes.bfloat16

    def pack_hidden(w, b):
        """[E,200,200] + [E,200] -> Ka [E,128,256], Kb [E,73,256].

        Layout: [Ma(cols 0:128) | Mb(cols 128:256)]; Mb cols 0:72 are
        features 128:200, col 72 is the ones-regeneration lane, rest 0.
        Kb rows 0:72 are input features 128:200, row 72 is [bias | v*].
        """
        wf = np.asarray(w, f)
        bl = np.asarray(b, f).reshape(E, H)
        ka = np.zeros((E, K0, 256), f)
        kb = np.zeros((E, K1, 256), f)
        ka[:, :, 0:128] = wf[:, 0:128, 0:128]
        ka[:, :, 128:200] = wf[:, 0:128, 128:200]
        kb[:, 0:72, 0:128] = wf[:, 128:200, 0:128]
        kb[:, 0:72, 128:200] = wf[:, 128:200, 128:200]
        kb[:, 72, 0:128] = bl[:, 0:128]
        kb[:, 72, 128:200] = bl[:, 128:200]
        kb[:, 72, 200] = VSTAR  # ones lane: silu(VSTAR * 1) == 1
        # NOTE: Mb column indices 128+j hold feature 128+j's output; the
        # ones lane is Mb col 200-128=72 -> absolute col 200.
        return ka, kb

    # layer 0: [E, 39, 256] with ones row 38; Mb col 72 (abs 200) = VSTAR
    w0f = np.asarray(w0, f)
    b0f = np.asarray(b0, f).reshape(E, H)
    w0p = np.zeros((E, INP, 256), f)
    w0p[:, 0:IN_DIM, 0:128] = w0f[:, :, 0:128]
    w0p[:, 0:IN_DIM, 128:200] = w0f[:, :, 128:200]
    w0p[:, IN_DIM, 0:128] = b0f[:, 0:128]
    w0p[:, IN_DIM, 128:200] = b0f[:, 128:200]
    w0p[:, IN_DIM, 200] = VSTAR

    w1a, w1b = pack_hidden(w1, b1)
    w2a, w2b = pack_hidden(w2, b2)
    w3a, w3b = pack_hidden(w3, b3)

    # layer 4: [mean(31) | pad | logvar(31)], bias row included
    b4f = np.asarray(b4, f).reshape(E, 2 * OUT)
    w4f = np.asarray(w4, f)
    w4a = np.zeros((E, K0, M4), f)
    w4b = np.zeros((E, K1, M4), f)
    w4a[:, :, 0:OUT] = w4f[:, 0:128, 0:OUT]
    w4a[:, :, OUT + 1 : M4] = w4f[:, 0:128, OUT : 2 * OUT]
    w4b[:, 0:72, 0:OUT] = w4f[:, 128:200, 0:OUT]
    w4b[:, 0:72, OUT + 1 : M4] = w4f[:, 128:200, OUT : 2 * OUT]
    w4b[:, 72, 0:OUT] = b4f[:, 0:OUT]
    w4b[:, 72, OUT + 1 : M4] = b4f[:, OUT : 2 * OUT]

    common = {
        "w0p": np.ascontiguousarray(w0p.astype(bf)),
        "w1a": np.ascontiguousarray(w1a.astype(bf)),
        "w1b": np.ascontiguousarray(w1b.astype(bf)),
        "w2a": np.ascontiguousarray(w2a.astype(bf)),
        "w2b": np.ascontiguousarray(w2b.astype(bf)),
        "w3a": np.ascontiguousarray(w3a.astype(bf)),
        "w3b": np.ascontiguousarray(w3b.astype(bf)),
        "w4a": np.ascontiguousarray(w4a.astype(bf)),
        "w4b": np.ascontiguousarray(w4b.astype(bf)),
    }
    mx = np.asarray(max_logvar, f).reshape(OUT)
    mn = np.asarray(min_logvar, f).reshape(OUT)
    c2 = 1.0 + np.exp(mx - mn)  # [31]

    def _pack31(v, pad=0.0):  # [31] -> [PACK*32, 1] with pad rows
        out = np.full((PACK, RSTRIDE), pad, f)
        out[:, :OUT] = v[None, :]
        return out.reshape(P2P, 1)

    common["c1"] = np.ascontiguousarray(_pack31(mx))
    common["c2"] = np.ascontiguousarray(_pack31(c2, pad=1.0))
    common["minlv"] = np.ascontiguousarray(_pack31(mn))

    xf = np.asarray(x, f)
    in_maps = []
    for c in range(NCORES):
        xc = np.empty((E, INP, BS), f)
        xc[:, 0:IN_DIM, :] = xf[:, c * BS : (c + 1) * BS, :].transpose(0, 2, 1)
        xc[:, IN_DIM, :] = 1.0
        in_maps.append({"xT": np.ascontiguousarray(xc.astype(bf)), **common})
    return in_maps


def _run(inputs, trace=False):
    from concourse.bass_utils import run_bass_kernel_spmd

    if "nc" not in _CACHE:
        _CACHE["nc"] = _build()
    nc = _CACHE["nc"]
    in_maps = _prep_host(**inputs)
    res = run_bass_kernel_spmd(nc, in_maps, core_ids=list(range(NCORES)), trace=trace)
    mean = np.concatenate(
        [res.results[c]["out_mean"].transpose(0, 2, 1) for c in range(NCORES)], axis=1
    )
    logvar = np.concatenate(
        [res.results[c]["out_logvar"].transpose(0, 2, 1) for c in range(NCORES)],
        axis=1,
    )
    return (mean, logvar), res


def kernel(**inputs):
    out, _ = _run(inputs, trace=False)
    return out


# revision 10
# speedup vs baseline: 1.7171x; 1.7171x over previous
"""Trainium2 Bass kernel for a 5-member ensemble dynamics MLP.

Model: per ensemble e, x[e] @ w0[e]+b0 -> silu -> (200x200 silu) x3 ->
w4[e]+b4 -> split (mean, logvar) -> double softplus clamp of logvar.

Sharding: pure data parallel over the batch dim (65536 -> 8 x 8192);
the ~1.4 MB of ensemble weights is replicated to every core.

v4 layout notes:
- All matmul operands are bfloat16; PSUM accumulates fp32. bf16 streams
  the PE at 1 cycle/row (fp32r is ~2x slower on HW).
- NT=512 with ONE merged [128, 1024] PSUM tile per layer: columns 0:512
  hold the M-block 0:128, columns 512:1024 hold features 128:200 (the
  stationary is zero-padded to M=128 so every PSUM row is written).
  One layer therefore costs one bias-free Silu over [128, 1024] on the
  scalar engine, and the 2-bank tiles give the psum pool 4 generations
  -> the PE runs ~2 layers ahead of the scalar engine and its p-state
  (1.2 -> 2.4 GHz after ~3us continuous) survives tile boundaries.
- All biases ride inside the matmuls: x is packed with a ones row, and
  each layer's K-block-b stationary carries [weights; bias row]. The
  ones lane regenerates itself through Silu via a weight v* with
  silu(v*) = 1 placed on the (ones-in -> ones-out) diagonal element.
- Output layer: W4' = [mean(31) | pad | logvar(31)] (single M=63 block,
  bias row included), so mean is a plain copy of PSUM 0:31 and raw
  logvar sits at 32:63 (both 32-aligned).
- DMA discipline: descriptor generation occupies the issuing queue, so
  inputs ride the sync queue (w0 -> x -> rest, per ensemble, so tile 0
  starts within ~5us) and outputs ride the vector queue (ordered after
  the DVE ops that produce them; no head-of-line blocking of the next
  ensemble's loads).
- logvar clamp (phase 2) uses the exact identity
    out = min + ln(C2 + t) - ln(1 + t),  t = e^{max - lv},
    C2 = 1 + e^{max - min}
  (one Exp + two Ln, one table set, + one DVE op), emitted inline per
  ensemble so the program-order scheduler keeps each run contiguous
  (~3 act-table switches per ensemble boundary).
- Raw logvar rows are staged packed 4-tiles-up (31 rows per 32-stride
  group) so phase-2 activations run ~124/128 full partitions.
"""

import sys

if "/opt/trn_rl_repo" not in sys.path:
    sys.path.insert(0, "/opt/trn_rl_repo")

import numpy as np

E = 5
B = 65536
IN_DIM = 38
INP = IN_DIM + 1  # +1 ones row for bias
H = 200
OUT = 31  # mean / logvar feature count
NCORES = 8
BS = B // NCORES  # samples per core
NT = 512  # batch-tile columns
NTILES = BS // NT
K0 = 128
K1 = H - K0 + 1  # 73: features 128:200 + ones/bias row
M4 = 2 * OUT + 1  # packed L4 output block: mean | pad | logvar
PACK = 4  # logvar tiles packed per partition group in phase 2
RSTRIDE = 32  # partition stride per packed tile
P2P = PACK * RSTRIDE  # 128 partitions, top row of each 32-group unused
P2N = 1024  # phase-2 Ln/DVE free-dim chunk
# silu(VSTAR) == 1.0: the ones lane regenerates itself through each layer
VSTAR = 1.2784645

_CACHE = {}


def _build():
    import concourse.bass as bass  # noqa: F401
    import concourse.tile as tile
    from concourse import bacc, mybir
    from contextlib import ExitStack

    fp32 = mybir.dt.float32
    bf16 = mybir.dt.bfloat16
    AF = mybir.ActivationFunctionType
    ALU = mybir.AluOpType

    nc = bacc.Bacc("TRN2", target_bir_lowering=False, debug=False)

    xT = nc.dram_tensor("xT", [E, INP, BS], bf16, kind="ExternalInput").ap()
    # stationary blocks, host-packed (bias rows + ones-regen included):
    #   wa[l]: [E, 128, 256] = K-block 0:128   -> [Ma(128) | Mb(128, padded)]
    #   wb[l]: [E, 73, 256]  = K-block 128:201 -> [Ma(128) | Mb(128, padded)]
    w0_d = nc.dram_tensor("w0p", [E, INP, 256], bf16, kind="ExternalInput").ap()
    wa_d = [
        nc.dram_tensor(f"w{l}a", [E, K0, 256], bf16, kind="ExternalInput").ap()
        for l in (1, 2, 3)
    ]
    wb_d = [
        nc.dram_tensor(f"w{l}b", [E, K1, 256], bf16, kind="ExternalInput").ap()
        for l in (1, 2, 3)
    ]
    w4a_d = nc.dram_tensor("w4a", [E, K0, M4], bf16, kind="ExternalInput").ap()
    w4b_d = nc.dram_tensor("w4b", [E, K1, M4], bf16, kind="ExternalInput").ap()
    # phase-2 per-partition constants, pre-tiled to the packed 128 rows
    c1_d = nc.dram_tensor("c1", [P2P, 1], fp32, kind="ExternalInput").ap()
    c2_d = nc.dram_tensor("c2", [P2P, 1], fp32, kind="ExternalInput").ap()
    minlv_d = nc.dram_tensor("minlv", [P2P, 1], fp32, kind="ExternalInput").ap()
    om_d = nc.dram_tensor("out_mean", [E, OUT, BS], fp32, kind="ExternalOutput").ap()
    ol_d = nc.dram_tensor("out_logvar", [E, OUT, BS], fp32, kind="ExternalOutput").ap()

    with tile.TileContext(nc) as tc, ExitStack() as ctx:
        wpool = ctx.enter_context(tc.tile_pool(name="wts", bufs=1))
        stpool = ctx.enter_context(tc.tile_pool(name="stage", bufs=1))
        xpool = ctx.enter_context(tc.tile_pool(name="x", bufs=2))
        hpool = ctx.enter_context(tc.tile_pool(name="h", bufs=4))
        pspool = ctx.enter_context(tc.tile_pool(name="ps", bufs=4, space="PSUM"))
        mpool = ctx.enter_context(tc.tile_pool(name="mean", bufs=1))
        tpool = ctx.enter_context(tc.tile_pool(name="p2t", bufs=2))
        p2pool = ctx.enter_context(tc.tile_pool(name="p2", bufs=2))

        W = {}

        def _const(tag, shape, src, dt=fp32):
            t = wpool.tile(shape, dt, tag=tag)
            nc.sync.dma_start(t[:], src)
            W[tag] = t
            return t

        # global phase-2 constants
        c1 = _const("c1", [P2P, 1], c1_d[:])
        c2 = _const("c2", [P2P, 1], c2_d[:])
        minlv = _const("minlv", [P2P, 1], minlv_d[:])

        # raw-logvar staging buffers, one per ensemble, packed 4-tiles-up
        stage = []
        for e in range(E):
            st = stpool.tile(
                [P2P, NTILES // PACK * NT], fp32, tag=f"stage_{e}", name=f"stage_{e}"
            )
            nc.vector.memset(st[:], 0.0)
            stage.append(st)

        ncol = NTILES // PACK * NT  # staged cols per ensemble

        def load_ensemble(e):
            """Queue ensemble e's input DMAs (sync ring), w0+x first."""
            _const(f"w0_{e}", [INP, 256], w0_d[e], bf16)
            xe = xpool.tile([INP, BS], bf16, tag="x")
            nc.sync.dma_start(xe[:], xT[e])
            for l in (1, 2, 3):
                _const(f"w{l}a_{e}", [K0, 256], wa_d[l - 1][e], bf16)
                _const(f"w{l}b_{e}", [K1, 256], wb_d[l - 1][e], bf16)
            _const(f"w4a_{e}", [K0, M4], w4a_d[e], bf16)
            _const(f"w4b_{e}", [K1, M4], w4b_d[e], bf16)
            return xe

        xe_next = load_ensemble(0)
        for e in range(E):
            xe = xe_next
            meanbuf = mpool.tile([OUT, BS], fp32, tag="meanbuf")

            # ---- MLP tiles (Silu table), two tiles software-pipelined ----
            # Engine streams execute in order, so interleaving two tiles
            # layer-by-layer makes the PE run MULTs(l, tB) while the scalar
            # engine runs Silu(l, tA): a 2-stage ping-pong pipeline.
            def mm_layer0(ps, cs):
                w0e = W[f"w0_{e}"]
                nc.tensor.matmul(
                    ps[:, 0:NT], w0e[:, 0:128], xe[:, cs], start=True, stop=True
                )
                nc.tensor.matmul(
                    ps[:, NT : 2 * NT],
                    w0e[:, 128:256],
                    xe[:, cs],
                    start=True,
                    stop=True,
                )

            def mm_hidden(l, ps, h):
                wa, wb = W[f"w{l}a_{e}"], W[f"w{l}b_{e}"]
                ha, hb = h[0:K0, 0:NT], h[0:K1, NT : 2 * NT]
                nc.tensor.matmul(ps[:, 0:NT], wa[:, 0:128], ha, start=True, stop=False)
                nc.tensor.matmul(ps[:, 0:NT], wb[:, 0:128], hb, start=False, stop=True)
                nc.tensor.matmul(
                    ps[:, NT : 2 * NT], wa[:, 128:256], ha, start=True, stop=False
                )
                nc.tensor.matmul(
                    ps[:, NT : 2 * NT], wb[:, 128:256], hb, start=False, stop=True
                )

            def silu(ps):
                h = hpool.tile([K0, 2 * NT], bf16, tag="h")
                nc.scalar.activation(h[:], ps[:], AF.Silu)
                return h

            def mm_out(h):
                ha, hb = h[0:K0, 0:NT], h[0:K1, NT : 2 * NT]
                pm = pspool.tile([M4, NT], fp32, tag="ps")
                nc.tensor.matmul(pm[:], W[f"w4a_{e}"][:], ha, start=True, stop=False)
                nc.tensor.matmul(pm[:], W[f"w4b_{e}"][:], hb, start=False, stop=True)
                return pm

            def tail_out(t, pm):
                cs = slice(t * NT, (t + 1) * NT)
                nc.vector.tensor_copy(meanbuf[:, cs], pm[0:OUT, :])
                # stash raw logvar: tile t -> rows 32*(t%4), cols NT*(t//4)
                r = (t % PACK) * RSTRIDE
                c = (t // PACK) * NT
                nc.vector.tensor_copy(
                    stage[e][r : r + OUT, c : c + NT], pm[OUT + 1 : M4, :]
                )

            for tp in range(NTILES // 2):
                tA, tB = 2 * tp, 2 * tp + 1
                csA = slice(tA * NT, (tA + 1) * NT)
                csB = slice(tB * NT, (tB + 1) * NT)
                psA = pspool.tile([K0, 2 * NT], fp32, tag="ps")
                mm_layer0(psA, csA)
                psB = pspool.tile([K0, 2 * NT], fp32, tag="ps")
                mm_layer0(psB, csB)
                hA = silu(psA)
                hB = silu(psB)
                for l in (1, 2, 3):
                    psA = pspool.tile([K0, 2 * NT], fp32, tag="ps")
                    mm_hidden(l, psA, hA)
                    psB = pspool.tile([K0, 2 * NT], fp32, tag="ps")
                    mm_hidden(l, psB, hB)
                    hA = silu(psA)
                    hB = silu(psB)
                pmA = mm_out(hA)
                pmB = mm_out(hB)
                tail_out(tA, pmA)
                tail_out(tB, pmB)

            # prefetch next ensemble's inputs into the sync ring BEFORE this
            # ensemble's outputs, so the late-ready output DMAs can't
            # head-of-line-block the loads
            if e + 1 < E:
                xe_next = load_ensemble(e + 1)
            nc.sync.dma_start(om_d[e], meanbuf[:])

            # ---- logvar clamp (Exp/Ln table), inline per ensemble ----
            #   t   = Exp(-z + c1) = e^{max - lv}   (c1 = max; b4lv in matmul)
            #   out = min + Ln(t + C2) - Ln(t + 1),  C2 = 1 + e^{max - min}
            te = tpool.tile([P2P, ncol], fp32, tag="p2t")
            nc.scalar.activation(te[:], stage[e][:], AF.Exp, bias=c1[:], scale=-1.0)
            for g in range(ncol // P2N):
                gs = slice(g * P2N, (g + 1) * P2N)
                a = p2pool.tile([P2P, P2N], fp32, tag="p2a")
                nc.scalar.activation(a[:], te[:, gs], AF.Ln, bias=c2[:])
                b = p2pool.tile([P2P, P2N], fp32, tag="p2b")
                nc.scalar.activation(b[:], te[:, gs], AF.Ln, bias=1.0)
                lvo = p2pool.tile([P2P, P2N], fp32, tag="p2c")
                # (a + min) - b
                nc.vector.scalar_tensor_tensor(
                    lvo[:], a[:], minlv[:], b[:], ALU.add, ALU.subtract
                )
                # unpack to [E, 31, BS] via the vector DMA queue
                for j in range(P2N // NT):
                    tcol = g * (P2N // NT) + j  # global col-block = t // PACK
                    for r in range(PACK):
                        t = tcol * PACK + r
                        nc.sync.dma_start(
                            ol_d[e, :, t * NT : (t + 1) * NT],
                            lvo[r * RSTRIDE : r * RSTRIDE + OUT, j * NT : (j + 1) * NT],
                        )

    nc.compile()
    return nc


def _prep_host(x, w0, b0, w1, b1, w2, b2, w3, b3, w4, b4, max_logvar, min_logvar):
    import ml_dtypes

    f = np.float32
    bf = ml_dtypes.bfloat16

    def pack_hidden(w, b):
        """[E,200,200] + [E,200] -> Ka [E,128,256], Kb [E,73,256].

        Layout: [Ma(cols 0:128) | Mb(cols 128:256)]; Mb cols 0:72 are
        features 128:200, col 72 is the ones-regeneration lane, rest 0.
        Kb rows 0:72 are input features 128:200, row 72 is [bias | v*].
        """
        wf = np.asarray(w, f)
        bl = np.asarray(b, f).reshape(E, H)
        ka = np.zeros((E, K0, 256), f)
        kb = np.zeros((E, K1, 256), f)
        ka[:, :, 0:128] = wf[:, 0:128, 0:128]
        ka[:, :, 128:200] = wf[:, 0:128, 128:200]
        kb[:, 0:72, 0:128] = wf[:, 128:200, 0:128]
        kb[:, 0:72, 128:200] = wf[:, 128:200, 128:200]
        kb[:, 72, 0:128] = bl[:, 0:128]
        kb[:, 72, 128:200] = bl[:, 128:200]
        kb[:, 72, 200] = VSTAR  # ones lane: silu(VSTAR * 1) == 1
        # NOTE: Mb column indices 128+j hold feature 128+j's output; the
        # ones lane is Mb col 200-128=72 -> absolute col 200.
        return ka, kb

    # layer 0: [E, 39, 256] with ones row 38; Mb col 72 (abs 200) = VSTAR
    w0f = np.asarray(w0, f)
    b0f = np.asarray(b0, f).reshape(E, H)
    w0p = np.zeros((E, INP, 256), f)
    w0p[:, 0:IN_DIM, 0:128] = w0f[:, :, 0:128]
    w0p[:, 0:IN_DIM, 128:200] = w0f[:, :, 128:200]
    w0p[:, IN_DIM, 0:128] = b0f[:, 0:128]
    w0p[:, IN_DIM, 128:200] = b0f[:, 128:200]
    w0p[:, IN_DIM, 200] = VSTAR

    w1a, w1b = pack_hidden(w1, b1)
    w2a, w2b = pack_hidden(w2, b2)
    w3a, w3b = pack_hidden(w3, b3)

    # layer 4: [mean(31) | pad | logvar(31)], bias row included
    b4f = np.asarray(b4, f).reshape(E, 2 * OUT)
    w4f = np.asarray(w4, f)
    w4a = np.zeros((E, K0, M4), f)
    w4b = np.zeros((E, K1, M4), f)
    w4a[:, :, 0:OUT] = w4f[:, 0:128, 0:OUT]
    w4a[:, :, OUT + 1 : M4] = w4f[:, 0:128, OUT : 2 * OUT]
    w4b[:, 0:72, 0:OUT] = w4f[:, 128:200, 0:OUT]
    w4b[:, 0:72, OUT + 1 : M4] = w4f[:, 128:200, OUT : 2 * OUT]
    w4b[:, 72, 0:OUT] = b4f[:, 0:OUT]
    w4b[:, 72, OUT + 1 : M4] = b4f[:, OUT : 2 * OUT]

    common = {
        "w0p": np.ascontiguousarray(w0p.astype(bf)),
        "w1a": np.ascontiguousarray(w1a.astype(bf)),
        "w1b": np.ascontiguousarray(w1b.astype(bf)),
        "w2a": np.ascontiguousarray(w2a.astype(bf)),
        "w2b": np.ascontiguousarray(w2b.astype(bf)),
        "w3a": np.ascontiguousarray(w3a.astype(bf)),
        "w3b": np.ascontiguousarray(w3b.astype(bf)),
        "w4a": np.ascontiguousarray(w4a.astype(bf)),
        "w4b": np.ascontiguousarray(w4b.astype(bf)),
    }
    mx = np.asarray(max_logvar, f).reshape(OUT)
    mn = np.asarray(min_logvar, f).reshape(OUT)
    c2 = 1.0 + np.exp(mx - mn)  # [31]

    def _pack31(v, pad=0.0):  # [31] -> [PACK*32, 1] with pad rows
        out = np.full((PACK, RSTRIDE), pad, f)
        out[:, :OUT] = v[None, :]
        return out.reshape(P2P, 1)

    common["c1"] = np.ascontiguousarray(_pack31(mx))
    common["c2"] = np.ascontiguousarray(_pack31(c2, pad=1.0))
    common["minlv"] = np.ascontiguousarray(_pack31(mn))

    xf = np.asarray(x, f)
    in_maps = []
    for c in range(NCORES):
        xc = np.empty((E, INP, BS), f)
        xc[:, 0:IN_DIM, :] = xf[:, c * BS : (c + 1) * BS, :].transpose(0, 2, 1)
        xc[:, IN_DIM, :] = 1.0
        in_maps.append({"xT": np.ascontiguousarray(xc.astype(bf)), **common})
    return in_maps


def _run(inputs, trace=False):
    from concourse.bass_utils import run_bass_kernel_spmd

    if "nc" not in _CACHE:
        _CACHE["nc"] = _build()
    nc = _CACHE["nc"]
    in_maps = _prep_host(**inputs)
    res = run_bass_kernel_spmd(nc, in_maps, core_ids=list(range(NCORES)), trace=trace)
    mean = np.concatenate(
        [res.results[c]["out_mean"].transpose(0, 2, 1) for c in range(NCORES)], axis=1
    )
    logvar = np.concatenate(
        [res.results[c]["out_logvar"].transpose(0, 2, 1) for c in range(NCORES)],
        axis=1,
    )
    return (mean, logvar), res


def kernel(**inputs):
    out, _ = _run(inputs, trace=False)
    return out


# revision 11
# speedup vs baseline: 2.0146x; 1.1733x over previous
"""Trainium2 Bass kernel for a 5-member ensemble dynamics MLP.

Model: per ensemble e, x[e] @ w0[e]+b0 -> silu -> (200x200 silu) x3 ->
w4[e]+b4 -> split (mean, logvar) -> double softplus clamp of logvar.

Sharding: pure data parallel over the batch dim (65536 -> 8 x 8192);
the ~1.4 MB of ensemble weights is replicated to every core.

v4 layout notes:
- All matmul operands are bfloat16; PSUM accumulates fp32. bf16 streams
  the PE at 1 cycle/row (fp32r is ~2x slower on HW).
- NT=512 with ONE merged [128, 1024] PSUM tile per layer: columns 0:512
  hold the M-block 0:128, columns 512:1024 hold features 128:200 (the
  stationary is zero-padded to M=128 so every PSUM row is written).
  One layer therefore costs one bias-free Silu over [128, 1024] on the
  scalar engine, and the 2-bank tiles give the psum pool 4 generations
  -> the PE runs ~2 layers ahead of the scalar engine and its p-state
  (1.2 -> 2.4 GHz after ~3us continuous) survives tile boundaries.
- All biases ride inside the matmuls: x is packed with a ones row, and
  each layer's K-block-b stationary carries [weights; bias row]. The
  ones lane regenerates itself through Silu via a weight v* with
  silu(v*) = 1 placed on the (ones-in -> ones-out) diagonal element.
- Output layer: W4' = [mean(31) | pad | logvar(31)] (single M=63 block,
  bias row included), so mean is a plain copy of PSUM 0:31 and raw
  logvar sits at 32:63 (both 32-aligned).
- DMA discipline: descriptor generation occupies the issuing queue, so
  inputs ride the sync queue (w0 -> x -> rest, per ensemble, so tile 0
  starts within ~5us) and outputs ride the vector queue (ordered after
  the DVE ops that produce them; no head-of-line blocking of the next
  ensemble's loads).
- logvar clamp (phase 2) uses the exact identity
    out = min + ln(C2 + t) - ln(1 + t),  t = e^{max - lv},
    C2 = 1 + e^{max - min}
  (one Exp + two Ln, one table set, + one DVE op), emitted inline per
  ensemble so the program-order scheduler keeps each run contiguous
  (~3 act-table switches per ensemble boundary).
- Raw logvar rows are staged packed 4-tiles-up (31 rows per 32-stride
  group) so phase-2 activations run ~124/128 full partitions.
"""

import sys

if "/opt/trn_rl_repo" not in sys.path:
    sys.path.insert(0, "/opt/trn_rl_repo")

import numpy as np

E = 5
B = 65536
IN_DIM = 38
INP = IN_DIM + 1  # +1 ones row for bias
H = 200
OUT = 31  # mean / logvar feature count
NCORES = 8
BS = B // NCORES  # samples per core
NT = 512  # batch-tile columns
NTILES = BS // NT
K0 = 128
K1 = H - K0 + 1  # 73: features 128:200 + ones/bias row
M4 = 2 * OUT + 1  # packed L4 output block: mean | pad | logvar
PACK = 4  # logvar tiles packed per partition group in phase 2
RSTRIDE = 32  # partition stride per packed tile
P2P = PACK * RSTRIDE  # 128 partitions, top row of each 32-group unused
P2N = 1024  # phase-2 Ln/DVE free-dim chunk
# silu(VSTAR) == 1.0: the ones lane regenerates itself through each layer
VSTAR = 1.2784645

_CACHE = {}


def _build():
    import concourse.bass as bass  # noqa: F401
    import concourse.tile as tile
    from concourse import bacc, mybir
    from contextlib import ExitStack

    fp32 = mybir.dt.float32
    bf16 = mybir.dt.bfloat16
    AF = mybir.ActivationFunctionType
    ALU = mybir.AluOpType

    nc = bacc.Bacc("TRN2", target_bir_lowering=False, debug=False)

    xT = nc.dram_tensor("xT", [E, INP, BS], bf16, kind="ExternalInput").ap()
    # stationary blocks, host-packed (bias rows + ones-regen included):
    #   wa[l]: [E, 128, 256] = K-block 0:128   -> [Ma(128) | Mb(128, padded)]
    #   wb[l]: [E, 73, 256]  = K-block 128:201 -> [Ma(128) | Mb(128, padded)]
    w0_d = nc.dram_tensor("w0p", [E, INP, 256], bf16, kind="ExternalInput").ap()
    wa_d = [
        nc.dram_tensor(f"w{l}a", [E, K0, 256], bf16, kind="ExternalInput").ap()
        for l in (1, 2, 3)
    ]
    wb_d = [
        nc.dram_tensor(f"w{l}b", [E, K1, 256], bf16, kind="ExternalInput").ap()
        for l in (1, 2, 3)
    ]
    w4a_d = nc.dram_tensor("w4a", [E, K0, M4], bf16, kind="ExternalInput").ap()
    w4b_d = nc.dram_tensor("w4b", [E, K1, M4], bf16, kind="ExternalInput").ap()
    # phase-2 per-partition constants, pre-tiled to the packed 128 rows
    c1_d = nc.dram_tensor("c1", [P2P, 1], fp32, kind="ExternalInput").ap()
    c2_d = nc.dram_tensor("c2", [P2P, 1], fp32, kind="ExternalInput").ap()
    minlv_d = nc.dram_tensor("minlv", [P2P, 1], fp32, kind="ExternalInput").ap()
    om_d = nc.dram_tensor("out_mean", [E, OUT, BS], fp32, kind="ExternalOutput").ap()
    ol_d = nc.dram_tensor(
        "out_logvar_raw", [E, P2P, NTILES // PACK * NT], fp32, kind="ExternalOutput"
    ).ap()

    with tile.TileContext(nc) as tc, ExitStack() as ctx:
        wpool = ctx.enter_context(tc.tile_pool(name="wts", bufs=1))
        stpool = ctx.enter_context(tc.tile_pool(name="stage", bufs=1))
        xpool = ctx.enter_context(tc.tile_pool(name="x", bufs=2))
        hpool = ctx.enter_context(tc.tile_pool(name="h", bufs=4))
        pspool = ctx.enter_context(tc.tile_pool(name="ps", bufs=4, space="PSUM"))
        mpool = ctx.enter_context(tc.tile_pool(name="mean", bufs=1))
        tpool = ctx.enter_context(tc.tile_pool(name="p2t", bufs=2))
        p2pool = ctx.enter_context(tc.tile_pool(name="p2", bufs=2))

        W = {}

        def _const(tag, shape, src, dt=fp32):
            t = wpool.tile(shape, dt, tag=tag)
            nc.sync.dma_start(t[:], src)
            W[tag] = t
            return t

        # global phase-2 constants
        c1 = _const("c1", [P2P, 1], c1_d[:])
        c2 = _const("c2", [P2P, 1], c2_d[:])
        minlv = _const("minlv", [P2P, 1], minlv_d[:])

        # raw-logvar staging buffers, one per ensemble, packed 4-tiles-up
        stage = []
        for e in range(E):
            st = stpool.tile(
                [P2P, NTILES // PACK * NT], fp32, tag=f"stage_{e}", name=f"stage_{e}"
            )
            nc.vector.memset(st[:], 0.0)
            stage.append(st)

        ncol = NTILES // PACK * NT  # staged cols per ensemble

        def load_ensemble(e):
            """Queue ensemble e's input DMAs (sync ring), w0+x first."""
            _const(f"w0_{e}", [INP, 256], w0_d[e], bf16)
            xe = xpool.tile([INP, BS], bf16, tag="x")
            nc.sync.dma_start(xe[:], xT[e])
            for l in (1, 2, 3):
                _const(f"w{l}a_{e}", [K0, 256], wa_d[l - 1][e], bf16)
                _const(f"w{l}b_{e}", [K1, 256], wb_d[l - 1][e], bf16)
            _const(f"w4a_{e}", [K0, M4], w4a_d[e], bf16)
            _const(f"w4b_{e}", [K1, M4], w4b_d[e], bf16)
            return xe

        xe_next = load_ensemble(0)
        for e in range(E):
            xe = xe_next
            meanbuf = mpool.tile([OUT, BS], fp32, tag="meanbuf")

            # ---- MLP tiles (Silu table), two tiles software-pipelined ----
            # Engine streams execute in order, so interleaving two tiles
            # layer-by-layer makes the PE run MULTs(l, tB) while the scalar
            # engine runs Silu(l, tA): a 2-stage ping-pong pipeline.
            def mm_layer0(ps, cs):
                w0e = W[f"w0_{e}"]
                nc.tensor.matmul(
                    ps[:, 0:NT], w0e[:, 0:128], xe[:, cs], start=True, stop=True
                )
                nc.tensor.matmul(
                    ps[:, NT : 2 * NT],
                    w0e[:, 128:256],
                    xe[:, cs],
                    start=True,
                    stop=True,
                )

            def mm_hidden(l, ps, h):
                wa, wb = W[f"w{l}a_{e}"], W[f"w{l}b_{e}"]
                ha, hb = h[0:K0, 0:NT], h[0:K1, NT : 2 * NT]
                nc.tensor.matmul(ps[:, 0:NT], wa[:, 0:128], ha, start=True, stop=False)
                nc.tensor.matmul(ps[:, 0:NT], wb[:, 0:128], hb, start=False, stop=True)
                nc.tensor.matmul(
                    ps[:, NT : 2 * NT], wa[:, 128:256], ha, start=True, stop=False
                )
                nc.tensor.matmul(
                    ps[:, NT : 2 * NT], wb[:, 128:256], hb, start=False, stop=True
                )

            def silu(ps):
                h = hpool.tile([K0, 2 * NT], bf16, tag="h")
                nc.scalar.activation(h[:], ps[:], AF.Silu)
                return h

            def mm_out(h):
                ha, hb = h[0:K0, 0:NT], h[0:K1, NT : 2 * NT]
                pm = pspool.tile([M4, NT], fp32, tag="ps")
                nc.tensor.matmul(pm[:], W[f"w4a_{e}"][:], ha, start=True, stop=False)
                nc.tensor.matmul(pm[:], W[f"w4b_{e}"][:], hb, start=False, stop=True)
                return pm

            def tail_out(t, pm):
                cs = slice(t * NT, (t + 1) * NT)
                nc.vector.tensor_copy(meanbuf[:, cs], pm[0:OUT, :])
                # stash raw logvar: tile t -> rows 32*(t%4), cols NT*(t//4)
                r = (t % PACK) * RSTRIDE
                c = (t // PACK) * NT
                nc.vector.tensor_copy(
                    stage[e][r : r + OUT, c : c + NT], pm[OUT + 1 : M4, :]
                )

            groups = [(0, 1, 2), (3, 4, 5), (6, 7, 8), (9, 10, 11), (12, 13), (14, 15)]
            for gi, grp in enumerate(groups):
                pss = []
                for t in grp:
                    ps = pspool.tile([K0, 2 * NT], fp32, tag="ps")
                    mm_layer0(ps, slice(t * NT, (t + 1) * NT))
                    pss.append(ps)
                hs = [silu(ps) for ps in pss]
                if gi == 0 and e + 1 < E:
                    # prefetch next ensemble's inputs into the sync ring early
                    xe_next = load_ensemble(e + 1)
                for l in (1, 2, 3):
                    pss = []
                    for h in hs:
                        ps = pspool.tile([K0, 2 * NT], fp32, tag="ps")
                        mm_hidden(l, ps, h)
                        pss.append(ps)
                    hs = [silu(ps) for ps in pss]
                pms = [mm_out(h) for h in hs]
                for t, pm in zip(grp, pms):
                    tail_out(t, pm)

            nc.sync.dma_start(om_d[e], meanbuf[:])

            # ---- logvar clamp (Exp/Ln table), inline per ensemble ----
            #   t   = Exp(-z + c1) = e^{max - lv}   (c1 = max; b4lv in matmul)
            #   out = min + Ln(t + C2) - Ln(t + 1),  C2 = 1 + e^{max - min}
            te = tpool.tile([P2P, ncol], fp32, tag="p2t")
            nc.scalar.activation(te[:], stage[e][:], AF.Exp, bias=c1[:], scale=-1.0)
            lvo = tpool.tile([P2P, ncol], fp32, tag="p2o")
            for g in range(ncol // P2N):
                gs = slice(g * P2N, (g + 1) * P2N)
                a = p2pool.tile([P2P, P2N], fp32, tag="p2a")
                nc.scalar.activation(a[:], te[:, gs], AF.Ln, bias=c2[:])
                b = p2pool.tile([P2P, P2N], fp32, tag="p2b")
                nc.scalar.activation(b[:], te[:, gs], AF.Ln, bias=1.0)
                # (a + min) - b
                nc.vector.scalar_tensor_tensor(
                    lvo[:, gs], a[:], minlv[:], b[:], ALU.add, ALU.subtract
                )
            # packed [128, ncol] out; host unpacks the 4x32-row tiling
            nc.sync.dma_start(ol_d[e], lvo[:])

    nc.compile()
    return nc


def _prep_host(x, w0, b0, w1, b1, w2, b2, w3, b3, w4, b4, max_logvar, min_logvar):
    import ml_dtypes

    f = np.float32
    bf = ml_dtypes.bfloat16

    def pack_hidden(w, b):
        """[E,200,200] + [E,200] -> Ka [E,128,256], Kb [E,73,256].

        Layout: [Ma(cols 0:128) | Mb(cols 128:256)]; Mb cols 0:72 are
        features 128:200, col 72 is the ones-regeneration lane, rest 0.
        Kb rows 0:72 are input features 128:200, row 72 is [bias | v*].
        """
        wf = np.asarray(w, f)
        bl = np.asarray(b, f).reshape(E, H)
        ka = np.zeros((E, K0, 256), f)
        kb = np.zeros((E, K1, 256), f)
        ka[:, :, 0:128] = wf[:, 0:128, 0:128]
        ka[:, :, 128:200] = wf[:, 0:128, 128:200]
        kb[:, 0:72, 0:128] = wf[:, 128:200, 0:128]
        kb[:, 0:72, 128:200] = wf[:, 128:200, 128:200]
        kb[:, 72, 0:128] = bl[:, 0:128]
        kb[:, 72, 128:200] = bl[:, 128:200]
        kb[:, 72, 200] = VSTAR  # ones lane: silu(VSTAR * 1) == 1
        # NOTE: Mb column indices 128+j hold feature 128+j's output; the
        # ones lane is Mb col 200-128=72 -> absolute col 200.
        return ka, kb

    # layer 0: [E, 39, 256] with ones row 38; Mb col 72 (abs 200) = VSTAR
    w0f = np.asarray(w0, f)
    b0f = np.asarray(b0, f).reshape(E, H)
    w0p = np.zeros((E, INP, 256), f)
    w0p[:, 0:IN_DIM, 0:128] = w0f[:, :, 0:128]
    w0p[:, 0:IN_DIM, 128:200] = w0f[:, :, 128:200]
    w0p[:, IN_DIM, 0:128] = b0f[:, 0:128]
    w0p[:, IN_DIM, 128:200] = b0f[:, 128:200]
    w0p[:, IN_DIM, 200] = VSTAR

    w1a, w1b = pack_hidden(w1, b1)
    w2a, w2b = pack_hidden(w2, b2)
    w3a, w3b = pack_hidden(w3, b3)

    # layer 4: [mean(31) | pad | logvar(31)], bias row included
    b4f = np.asarray(b4, f).reshape(E, 2 * OUT)
    w4f = np.asarray(w4, f)
    w4a = np.zeros((E, K0, M4), f)
    w4b = np.zeros((E, K1, M4), f)
    w4a[:, :, 0:OUT] = w4f[:, 0:128, 0:OUT]
    w4a[:, :, OUT + 1 : M4] = w4f[:, 0:128, OUT : 2 * OUT]
    w4b[:, 0:72, 0:OUT] = w4f[:, 128:200, 0:OUT]
    w4b[:, 0:72, OUT + 1 : M4] = w4f[:, 128:200, OUT : 2 * OUT]
    w4b[:, 72, 0:OUT] = b4f[:, 0:OUT]
    w4b[:, 72, OUT + 1 : M4] = b4f[:, OUT : 2 * OUT]

    common = {
        "w0p": np.ascontiguousarray(w0p.astype(bf)),
        "w1a": np.ascontiguousarray(w1a.astype(bf)),
        "w1b": np.ascontiguousarray(w1b.astype(bf)),
        "w2a": np.ascontiguousarray(w2a.astype(bf)),
        "w2b": np.ascontiguousarray(w2b.astype(bf)),
        "w3a": np.ascontiguousarray(w3a.astype(bf)),
        "w3b": np.ascontiguousarray(w3b.astype(bf)),
        "w4a": np.ascontiguousarray(w4a.astype(bf)),
        "w4b": np.ascontiguousarray(w4b.astype(bf)),
    }
    mx = np.asarray(max_logvar, f).reshape(OUT)
    mn = np.asarray(min_logvar, f).reshape(OUT)
    c2 = 1.0 + np.exp(mx - mn)  # [31]

    def _pack31(v, pad=0.0):  # [31] -> [PACK*32, 1] with pad rows
        out = np.full((PACK, RSTRIDE), pad, f)
        out[:, :OUT] = v[None, :]
        return out.reshape(P2P, 1)

    common["c1"] = np.ascontiguousarray(_pack31(mx))
    common["c2"] = np.ascontiguousarray(_pack31(c2, pad=1.0))
    common["minlv"] = np.ascontiguousarray(_pack31(mn))

    xf = np.asarray(x, f)
    in_maps = []
    for c in range(NCORES):
        xc = np.empty((E, INP, BS), f)
        xc[:, 0:IN_DIM, :] = xf[:, c * BS : (c + 1) * BS, :].transpose(0, 2, 1)
        xc[:, IN_DIM, :] = 1.0
        in_maps.append({"xT": np.ascontiguousarray(xc.astype(bf)), **common})
    return in_maps


def _run(inputs, trace=False):
    from concourse.bass_utils import run_bass_kernel_spmd

    if "nc" not in _CACHE:
        _CACHE["nc"] = _build()
    nc = _CACHE["nc"]
    in_maps = _prep_host(**inputs)
    res = run_bass_kernel_spmd(nc, in_maps, core_ids=list(range(NCORES)), trace=trace)
    mean = np.concatenate(
        [res.results[c]["out_mean"].transpose(0, 2, 1) for c in range(NCORES)], axis=1
    )
    ncol = NTILES // PACK * NT
    lvs = []
    for c in range(NCORES):
        raw = res.results[c]["out_logvar_raw"]  # [E, 128, ncol]
        r5 = raw.reshape(E, PACK, RSTRIDE, ncol // NT, NT)[:, :, :OUT]
        # (e, r, f, tcol, c) -> (e, tcol, r, c, f): col t*NT+c with t=tcol*PACK+r
        lvs.append(r5.transpose(0, 3, 1, 4, 2).reshape(E, BS, OUT))
    logvar = np.concatenate(lvs, axis=1)
    return (mean, logvar), res


def kernel(**inputs):
    out, _ = _run(inputs, trace=False)
    return out
